# revision 15
# baseline (speedup 1.0000x reference)
# Bass/Tile TRN2 kernel for nn_Conv1D_style: out = ((x * (cluster@style_L)) @ weight) * (cluster@style_R)
#
# Sharding: data-parallel over the batch dim. Each of the 8 cores gets a
# 1024-row slice of x/cluster and a full (replicated) weight/style_L/style_R.
#
# Per-core plan (M=1024 batch, K=4096 din, N=4096 dout), all matmuls bf16
# with fp32 PSUM accumulation:
#   aT[k] = xT[k] * (style_L[:, kslice].T @ clusterT)  -> bf16, SBUF-resident.
#   y[m,n] = sum_k aT[k][:, mslice].T @ W[k, nslice]   (32 accumulating MMs)
#   out[m,n] = y[m,n] * (clusterT[:, mslice].T @ style_R[:, nslice])
#
# The aT production is fused with the first n-block's accumulation (n=0,
# m=0..3 accumulate k-outer across 4 PSUM banks) so the PE never drains in
# the prologue. The K=64 style matmuls (tmpLT/tmpR) are row-packed two at a
# time via tile_position into the upper/lower 64 PE rows. ~50 warmup
# matmuls on a memset tile run during the initial DMA latency so the PE
# HAM clock-gate is at 8/8 when real matmuls start.
#
# DMA plan. The DMA fabric shares bandwidth between queues roughly
# per-packet, so a queue's share is proportional to its packet
# (per-partition-contiguous run) size; every startup transfer is therefore
# packed into >=4 KiB/partition blobs:
#  - Sync: five "boot" blobs carrying everything k0-11 needs, in
#    consumption order ([clT|sL k0-7], [x k0|W k0-1|x k1], [W k2-3|x k2-3],
#    [W k4-7|x k4-5], [x k6-7|W k8-11]), then W n0 k12-31 as one 2.5 MiB
#    20 KiB-packet DMA, then outputs. W prefetch for n>=1 is issued behind
#    an epilogue DMA so it streams only after the HBM-saturated prologue.
#  - Activation: the bulk x stream (k8+), self-paced by a 2-buf tile pool:
#    two dummy head allocations are consumed by tiny DVE copies placed
#    after the k1/k3 aT-muls, so each granule's descriptor fires only as
#    the prologue actually advances — it cannot crowd the boot blobs.
#  - GpSimd SWDGE: sL tail and sR as two big-packet DMAs, gated behind the
#    first boot blob by a tiny gpsimd copy.
# tmpR psum->sbuf staging runs on the otherwise-idle Activation engine and
# tr tiles are bf16. Output is written bf16 and upcast to fp32 on the host.

import numpy as np
import ml_dtypes

B, DIN, DOUT, NCL = 8192, 4096, 4096, 64
NCORES = 8
MB = B // NCORES          # batch rows per core
P = 128
NT = 512                  # n tile (dout cols per matmul)
KT = DIN // P             # 32 k tiles
MT = MB // P              # 8 m tiles
NTS = DOUT // NT          # 8 n tiles
FUSED = 4                 # m tiles of n=0 accumulated during the aT prologue
WARMUP = 50               # PE warmup matmuls (N=128) during startup DMA

# boot blob column offsets (bf16 elements per partition)
CL0 = 0          # clT                 [1024]
SL0 = 1024       # style_L k0-7       [1024]
XK0 = 2048       # x k0               [1024]
WK01 = 3072      # W n0 k0-1          [1024]
XK1 = 4096       # x k1               [1024]
WK23 = 5120      # W n0 k2-3          [1024]
XK23 = 6144      # x k2-3             [2048]
WK47 = 8192      # W n0 k4-7          [2048]
XK45 = 10240     # x k4-5             [2048]
XK67 = 12288     # x k6-7             [2048]
WK811 = 14336    # W n0 k8-11         [2048]
BOOTC = 16384

_CACHE = {}
LAST = {}                 # exposes the most recent BassKernelResults for test harnesses


def _build_program():
    import concourse.bacc as bacc
    import concourse.mybir as mybir
    import concourse.tile as tile

    bf16 = mybir.dt.bfloat16
    f32 = mybir.dt.float32

    nc = bacc.Bacc(None, target_bir_lowering=False, debug=False)

    # xT: [partition, k, batch] so any k-range is per-partition contiguous.
    # W: [n, partition, k, nt]. cluster/styles arrive duplicated: rows
    # 64-127 = rows 0-63 (row packing).
    boot_d = nc.declare_dram_parameter("boot", [P, BOOTC], bf16, isOutput=False)
    xT_d = nc.declare_dram_parameter("xT", [P, KT, MB], bf16, isOutput=False)
    w_d = nc.declare_dram_parameter("weight", [NTS, P, KT, NT], bf16, isOutput=False)
    sL_d = nc.declare_dram_parameter("style_L", [P, DIN - 8 * P], bf16, isOutput=False)
    sR_d = nc.declare_dram_parameter("style_R", [P, DOUT], bf16, isOutput=False)
    out_d = nc.declare_dram_parameter("out", [MB, DOUT], bf16, isOutput=True)

    H = NCL  # 64: row-pack halves

    with tile.TileContext(nc) as tc:
        with (
            tc.tile_pool(name="const", bufs=1) as const_pool,
            tc.tile_pool(name="atp", bufs=1) as at_pool,
            tc.tile_pool(name="wp", bufs=2) as w_pool,
            tc.tile_pool(name="xp", bufs=2) as x_pool,
            tc.tile_pool(name="evp", bufs=3) as ev_pool,
            # PSUM budget (8 banks): py 4 x [128,512] (tmpR psum + y
            # accumulators) + pl 2 x [128,1024] (2 banks each) = 8.
            tc.tile_pool(name="pyp", bufs=4, space="PSUM") as py_pool,
            tc.tile_pool(name="plp", bufs=2, space="PSUM") as pl_pool,
        ):
            boot = const_pool.tile([P, BOOTC], bf16, name="boot")
            sLr = const_pool.tile([P, DIN - 8 * P], bf16, name="sLr")
            sR = const_pool.tile([P, DOUT], bf16, name="sR")
            warm = const_pool.tile([P, P], bf16, name="warm")
            scr = const_pool.tile([P, 16], bf16, name="scr")
            nc.gpsimd.memset(warm[:], 0.5)

            # Boot blobs on Sync, consumption-ordered.
            for lo, hi in ((0, XK0), (XK0, WK23), (WK23, WK47),
                           (WK47, XK67), (XK67, BOOTC)):
                nc.sync.dma_start(boot[:, lo:hi], boot_d[:, lo:hi])

            # sL tail + sR on the GpSimd SWDGE queue, gated behind the first
            # boot blob so they don't contend with it.
            nc.gpsimd.tensor_copy(out=scr[0:1, :], in_=boot[0:1, 0:16])
            nc.gpsimd.dma_start(sLr[:], sL_d[:])
            nc.gpsimd.dma_start(sR[:], sR_d[:])

            # ---- PE warmup: keep the PE busy from program start until the
            # boot blob lands, so HAM un-throttles to 8/8 before real work.
            # Results go to a pl psum slot and are never read.
            wps = pl_pool.tile([P, MB], f32, name="wps", tag="pl")
            for i in range(WARMUP):
                nc.tensor.matmul(
                    wps[:, 0:P], warm[:], warm[:], start=True, stop=True
                )

            # W n0 k12-31: one 20 KiB-packet DMA on Sync behind the blobs.
            w0 = w_pool.tile([P, KT - 12, NT], bf16, name="w0", tag="wbig")
            nc.sync.dma_start(w0[:], w_d[0, :, 12:32, :])

            # Bulk x stream (k8+): dummy head allocations d0/d1 are consumed
            # by tiny DVE copies after the k1/k3 aT-muls (below), so with 2
            # pool bufs granule g's descriptor can only fire once the
            # prologue has consumed granule g-2.
            d0 = x_pool.tile([P, 16], bf16, name="d0", tag="xg")
            d1 = x_pool.tile([P, 16], bf16, name="d1", tag="xg")
            nc.gpsimd.memset(d0[0:1, :], 0.0)
            nc.gpsimd.memset(d1[0:1, :], 0.0)
            xgt = []
            for g in range(6):
                xg = x_pool.tile([P, 4, MB], bf16, name=f"xg{g}", tag="xg")
                nc.scalar.dma_start(xg[:], xT_d[:, 8 + 4 * g:12 + 4 * g, :])
                xgt.append(xg)

            def x_ap(k):
                if k == 0:
                    return boot[:, XK0:XK0 + MB]
                if k == 1:
                    return boot[:, XK1:XK1 + MB]
                if k < 4:
                    return boot[:, XK23 + (k - 2) * MB:XK23 + (k - 1) * MB]
                if k < 6:
                    return boot[:, XK45 + (k - 4) * MB:XK45 + (k - 3) * MB]
                if k < 8:
                    return boot[:, XK67 + (k - 6) * MB:XK67 + (k - 5) * MB]
                return xgt[(k - 8) // 4][:, (k - 8) % 4, :]

            def w0_ap(k):
                if k < 2:
                    return boot[:, WK01 + k * NT:WK01 + (k + 1) * NT]
                if k < 4:
                    return boot[:, WK23 + (k - 2) * NT:WK23 + (k - 1) * NT]
                if k < 8:
                    return boot[:, WK47 + (k - 4) * NT:WK47 + (k - 3) * NT]
                if k < 12:
                    return boot[:, WK811 + (k - 8) * NT:WK811 + (k - 7) * NT]
                return w0[:, k - 12, :]

            def sL_ap(rows, k):
                if k < 8:
                    return boot[rows, SL0 + k * P:SL0 + (k + 1) * P]
                return sLr[rows, (k - 8) * P:(k - 8 + 1) * P]

            def tmpr_pair(n, m, psum_src="py"):
                """Row-packed pair: tmpR tiles for (m, m+1) at n, staged to SBUF.

                psum_src="pl" borrows a pl-pool tile (two banks) instead of two
                py slots — required in the fused prologue where all four py
                slots are held by the open accumulators.
                """
                if psum_src == "pl":
                    prp = pl_pool.tile([P, MB], f32, name=f"prf{n}_{m}", tag="pl")
                    pra, prb = prp[:, 0:NT], prp[:, NT:MB]
                else:
                    pra = py_pool.tile([P, NT], f32, name=f"pr{n}_{m}", tag="py")
                    prb = py_pool.tile([P, NT], f32, name=f"pr{n}_{m + 1}", tag="py")
                nc.tensor.matmul(
                    pra[:],
                    boot[:H, CL0 + m * P:CL0 + (m + 1) * P],
                    sR[:H, n * NT:(n + 1) * NT],
                    start=True, stop=True, tile_position=(0, 0),
                )
                nc.tensor.matmul(
                    prb[:],
                    boot[H:, CL0 + (m + 1) * P:CL0 + (m + 2) * P],
                    sR[H:, n * NT:(n + 1) * NT],
                    start=True, stop=True, tile_position=(H, 0),
                )
                tra = ev_pool.tile([P, NT], bf16, name=f"tr{n}_{m}", tag="tr", bufs=6)
                trb = ev_pool.tile([P, NT], bf16, name=f"tr{n}_{m + 1}", tag="tr", bufs=6)
                # psum->sbuf staging on the (otherwise idle) Activation
                # engine: keeps the DVE free for the prologue's aT muls
                nc.scalar.copy(out=tra[:], in_=pra[:])
                nc.scalar.copy(out=trb[:], in_=prb[:])
                return tra, trb

            def epilogue(n, m, py, tr, split=False):
                ot = ev_pool.tile([P, NT], bf16, name=f"ot{n}_{m}", tag="ot")
                if split:
                    hw = NT // 2
                    for h in range(2):
                        s = slice(h * hw, (h + 1) * hw)
                        nc.vector.tensor_mul(out=ot[:, s], in0=py[:, s], in1=tr[:, s])
                        nc.sync.dma_start(
                            out_d[m * P:(m + 1) * P,
                                  n * NT + h * hw:n * NT + (h + 1) * hw],
                            ot[:, s],
                        )
                else:
                    nc.vector.tensor_mul(out=ot[:], in0=py[:], in1=tr[:])
                    nc.sync.dma_start(
                        out_d[m * P:(m + 1) * P, n * NT:(n + 1) * NT], ot[:]
                    )

            # ---- fused prologue: aT production + n0/m0..3 k-outer accumulation ----
            py_f = [
                py_pool.tile([P, NT], f32, name=f"py0_{m}", tag="py")
                for m in range(FUSED)
            ]
            at_tiles = []
            tr_f = []
            for k in range(KT):
                # tmpLT: row-packed pair, both batch halves in one slot
                pl = pl_pool.tile([P, MB], f32, name=f"pl{k}", tag="pl")
                nc.tensor.matmul(
                    pl[:, 0:NT],
                    sL_ap(slice(0, H), k),
                    boot[:H, CL0:CL0 + NT],
                    start=True, stop=True, tile_position=(0, 0),
                )
                nc.tensor.matmul(
                    pl[:, NT:MB],
                    sL_ap(slice(H, P), k),
                    boot[H:, CL0 + NT:CL0 + MB],
                    start=True, stop=True, tile_position=(H, 0),
                )
                at_k = at_pool.tile([P, MB], bf16, name=f"at{k}", tag=f"at{k}")
                nc.vector.tensor_mul(out=at_k[:], in0=x_ap(k), in1=pl[:])
                at_tiles.append(at_k)
                if k == 1:
                    nc.vector.tensor_copy(out=scr[0:1, :], in_=d0[0:1, :])
                elif k == 3:
                    nc.vector.tensor_copy(out=scr[0:1, :], in_=d1[0:1, :])
                for m in range(FUSED):
                    nc.tensor.matmul(
                        py_f[m][:],
                        at_k[:, m * P:(m + 1) * P],
                        w0_ap(k),
                        start=(k == 0), stop=(k == KT - 1),
                    )
                if k == 15:
                    # tmpR for the fused m tiles; sR arrives on the SWDGE
                    # queue well before this point
                    tr_f += tmpr_pair(0, 0, psum_src="pl")
                elif k == 23:
                    tr_f += tmpr_pair(0, 2, psum_src="pl")
            for m in range(FUSED):
                epilogue(0, m, py_f[m], tr_f[m])

            # ---- standard m-pair body: two 32-MM groups with the packed tmpR
            # pair injected mid-group (the deep MM pipeline hides its
            # LDWEIGHTS; at a group boundary it costs a full extra slot) ----
            def body_pair(n, m, w_ap, last=False):
                tra = trb = None
                for mm in (m, m + 1):
                    py = py_pool.tile([P, NT], f32, name=f"py{n}_{mm}", tag="py")
                    for k in range(KT):
                        nc.tensor.matmul(
                            py[:],
                            at_tiles[k][:, mm * P:(mm + 1) * P],
                            w_ap(k),
                            start=(k == 0), stop=(k == KT - 1),
                        )
                        if mm == m and k == KT // 2:
                            tra, trb = tmpr_pair(n, m)
                    epilogue(n, mm, py, tra if mm == m else trb,
                             split=(last and mm == m + 1))

            # W prefetch for n>=1: issue the descriptors behind an epilogue
            # DMA on the Sync queue so the stream starts only once the
            # prologue (which saturates HBM) has finished; prefetch exactly
            # one n ahead (w_pool bufs=2).
            wn_tiles = {}
            for n in range(1, NTS):
                wn_tiles[n] = w_pool.tile([P, KT, NT], bf16, name=f"w{n}", tag="wbig")

            def fetch_w(n):
                nc.sync.dma_start(wn_tiles[n][:, 0:16, :], w_d[n, :, 0:16, :])
                nc.sync.dma_start(wn_tiles[n][:, 16:32, :], w_d[n, :, 16:32, :])

            fetch_w(1)
            body_pair(0, 4, w0_ap)
            body_pair(0, 6, w0_ap)
            # n = 1..7
            for n in range(1, NTS):
                if n + 1 < NTS:
                    fetch_w(n + 1)
                wt = wn_tiles[n]
                w_ap = lambda k, wt=wt: wt[:, k, :]
                for m in range(0, MT, 2):
                    body_pair(n, m, w_ap, last=(n == NTS - 1 and m == MT - 2))

    nc.finalize()
    return nc


def _get_program():
    if "nc" not in _CACHE:
        _CACHE["nc"] = _build_program()
    return _CACHE["nc"]


def kernel(x, cluster, weight, style_L, style_R):
    import os

    # The NTFF trace path needs an antenv hook this container lacks; never
    # let a stray BASS_TRACE env take the run down that path.
    os.environ.setdefault("BASS_NEVER_TRACE", "1")
    from concourse.bass_utils import run_bass_kernel_spmd

    nc = _get_program()
    bf16 = ml_dtypes.bfloat16

    # W: [din, dout] -> [n, p, k, nt] partition-major for contiguous DMA
    w_bf = np.asarray(weight, dtype=np.float32).astype(bf16)
    w_r = np.ascontiguousarray(
        w_bf.reshape(KT, P, NTS, NT).transpose(2, 1, 0, 3)
    )
    # styles/cluster duplicated across both 64-row halves for row packing
    sL1 = np.asarray(style_L, dtype=np.float32).astype(bf16)
    sR1 = np.asarray(style_R, dtype=np.float32).astype(bf16)
    sL = np.ascontiguousarray(np.vstack([sL1, sL1]))
    sR = np.ascontiguousarray(np.vstack([sR1, sR1]))
    sL_tail = np.ascontiguousarray(sL[:, 8 * P:])

    def wk(lo, hi):  # W n0 k-range as [P, (hi-lo)*NT]
        return w_r[0][:, lo:hi, :].reshape(P, (hi - lo) * NT)

    in_maps = []
    for c in range(NCORES):
        xs = np.asarray(x[c * MB:(c + 1) * MB], dtype=np.float32)
        xT = np.ascontiguousarray(xs.T).astype(bf16)          # [DIN, MB]
        xT_r = np.ascontiguousarray(
            xT.reshape(KT, P, MB).transpose(1, 0, 2)          # [P, KT, MB]
        )
        clT1 = np.ascontiguousarray(
            np.asarray(cluster[c * MB:(c + 1) * MB], dtype=np.float32).T
        ).astype(bf16)
        clT = np.ascontiguousarray(np.vstack([clT1, clT1]))
        xk = lambda lo, hi: xT_r[:, lo:hi, :].reshape(P, (hi - lo) * MB)
        boot = np.ascontiguousarray(np.concatenate(
            [clT, sL[:, 0:8 * P], xk(0, 1), wk(0, 2), xk(1, 2), wk(2, 4),
             xk(2, 4), wk(4, 8), xk(4, 6), xk(6, 8), wk(8, 12)], axis=1))
        assert boot.shape == (P, BOOTC)
        in_maps.append(
            {"boot": boot, "xT": xT_r, "weight": w_r,
             "style_L": sL_tail, "style_R": sR}
        )

    res = run_bass_kernel_spmd(nc, in_maps, list(range(NCORES)))
    LAST["results"] = res
    LAST["in_maps"] = in_maps
    out = np.concatenate(
        [np.asarray(res.results[c]["out"], dtype=np.float32) for c in range(NCORES)],
        axis=0,
    )
    return out
